# revision 1
# baseline (speedup 1.0000x reference)
"""Causal multi-head attention block (QKV proj + SDPA + out proj) on 8 TRN2 cores.

Sharding: batch (4) x head-group (2 groups of 8 heads). Core c handles batch
c//2, heads [g*8, g*8+8) with g = c%2. Each core:
  - projects x[b] to q,k,v for its 8 heads (bf16 matmuls, fp32 accum)
  - computes causal attention in the S^T orientation (k on partitions), with
    softmax denominators obtained via a fused ones-row in the PV matmul
  - computes the partial output projection against its slice of Wo
  - pair-wise ReduceScatter sums the two head-groups' partials and leaves each
    core with one L-half of the final output for its batch.

All heavy matmuls are bf16 with fp32 PSUM accumulation. Hardcoded shapes per
the problem spec: x [4, 2048, 1024], 16 heads, head_dim 64.
"""
import sys
import os

if '/opt/trn_rl_repo' not in sys.path:
    sys.path.insert(0, '/opt/trn_rl_repo')

import numpy as np
import ml_dtypes

import concourse.bass as bass
import concourse.mybir as mybir
import concourse.tile as tile
from concourse import bacc
from concourse.bass_utils import run_bass_kernel_spmd
from concourse.tile import TileContext

bf16 = ml_dtypes.bfloat16
F32 = mybir.dt.float32
BF16 = mybir.dt.bfloat16

B, L, D, H, HD = 4, 2048, 1024, 16, 64
HPC = 8           # heads per core
GD = HPC * HD     # 512 dims per head-group
LH = L // 2       # 1024, L-half owned by each core after ReduceScatter

_CACHE = {}


def _build_nc(stage="full"):
    nc = bacc.Bacc("TRN2", target_bir_lowering=False, debug=False, num_devices=8)

    xT_d = nc.dram_tensor("xT", [D, L], BF16, kind="ExternalInput").ap()
    wT_d = nc.dram_tensor("wT", [D, 3 * GD], BF16, kind="ExternalInput").ap()
    woT_d = nc.dram_tensor("woT", [GD, D], BF16, kind="ExternalInput").ap()
    bqk_d = nc.dram_tensor("bqk", [2 * GD, 1], F32, kind="ExternalInput").ap()
    bv_d = nc.dram_tensor("bv", [128, GD], F32, kind="ExternalInput").ap()
    bob_d = nc.dram_tensor("bob", [128, D], F32, kind="ExternalInput").ap()
    masks_d = nc.dram_tensor("masks", [128, 4 * 512], BF16, kind="ExternalInput").ap()
    y_d = nc.dram_tensor("y", [LH, D], F32, kind="ExternalOutput").ap()
    dbg_d = (nc.dram_tensor("dbgOT", [512, L], F32, kind="ExternalOutput").ap()
             if stage == "norsc_dbg" else None)

    with TileContext(nc) as tc:
        with (
            tc.tile_pool(name="persist", bufs=1) as persist,
            tc.tile_pool(name="exps", bufs=3) as exps_pool,
            tc.tile_pool(name="small", bufs=2) as small,
            tc.tile_pool(name="ystage", bufs=3) as ystage,
            tc.tile_pool(name="ps_s", bufs=1, space="PSUM") as ps_s,
            tc.tile_pool(name="ps_o", bufs=1, space="PSUM") as ps_o,
            tc.tile_pool(name="ps_op", bufs=2, space="PSUM") as ps_op,
            tc.tile_pool(name="dram", bufs=1, space="DRAM") as dram,
        ):
            # ---- persistent SBUF tensors -------------------------------------
            xT = [persist.tile([128, L], BF16, tag=f"xT{c}", name=f"xT{c}") for c in range(8)]
            wT = [persist.tile([128, 3 * GD], BF16, tag=f"wT{c}", name=f"wT{c}") for c in range(8)]
            qkT = [persist.tile([128, L], BF16, tag=f"qkT{i}", name=f"qkT{i}") for i in range(8)]
            Vt = [persist.tile([128, HPC * (HD + 1)], BF16, tag=f"V{i}", name=f"V{i}")
                  for i in range(16)]
            OTn = [persist.tile([128, L], BF16, tag=f"OTn{p}", name=f"OTn{p}") for p in range(4)]
            woT = [persist.tile([128, D], BF16, tag=f"woT{p}", name=f"woT{p}") for p in range(4)]
            bqk = persist.tile([128, 8], F32, tag="bqk")
            bv = persist.tile([128, GD], F32, tag="bv")
            bob = persist.tile([128, D], F32, tag="bob")
            masks = persist.tile([128, 4 * 512], BF16, tag="masks")

            # ---- input loads -------------------------------------------------
            for c in range(8):
                nc.sync.dma_start(out=wT[c], in_=wT_d[c * 128:(c + 1) * 128, :])
                nc.sync.dma_start(out=xT[c], in_=xT_d[c * 128:(c + 1) * 128, :])
            for p in range(4):
                nc.sync.dma_start(out=woT[p], in_=woT_d[p * 128:(p + 1) * 128, :])
            for dt in range(8):
                nc.sync.dma_start(out=bqk[:, dt:dt + 1],
                                  in_=bqk_d[dt * 128:(dt + 1) * 128, :])
            nc.sync.dma_start(out=bv, in_=bv_d[:, :])
            nc.sync.dma_start(out=bob, in_=bob_d[:, :])
            nc.sync.dma_start(out=masks, in_=masks_d[:, :])

            # ---- projection: qkT[dt] = (Wqk x^T + bqk) as bf16 ---------------
            # emitted pair-major so attention for pair p can start early
            for p in range(4):
                for dt in (p, 4 + p):          # q-pair tile, then k-pair tile
                    for lsb in range(4):
                        ps = ps_op.tile([128, 512], F32, name="ps_proj")
                        for c in range(8):
                            nc.tensor.matmul(
                                ps[:],
                                lhsT=wT[c][:, dt * 128:(dt + 1) * 128],
                                rhs=xT[c][:, lsb * 512:(lsb + 1) * 512],
                                start=(c == 0), stop=(c == 7),
                            )
                        nc.scalar.activation(
                            qkT[dt][:, lsb * 512:(lsb + 1) * 512], ps[:],
                            mybir.ActivationFunctionType.Identity,
                            bias=bqk[:, dt:dt + 1],
                        )

            # ---- projection: V~ tiles [128, 8*(64+1)] with fused ones col ----
            for lb in range(16):
                ps = ps_op.tile([128, 512], F32, name="ps_proj")
                for c in range(8):
                    nc.tensor.matmul(
                        ps[:],
                        lhsT=xT[c][:, lb * 128:(lb + 1) * 128],
                        rhs=wT[c][:, 1024:1536],
                        start=(c == 0), stop=(c == 7),
                    )
                v_grp = Vt[lb][:].rearrange("p (h c) -> p h c", c=HD + 1)
                nc.vector.tensor_add(
                    v_grp[:, :, 0:HD],
                    ps[:].rearrange("p (h c) -> p h c", c=HD),
                    bv[:].rearrange("p (h c) -> p h c", c=HD),
                )
                nc.vector.memset(v_grp[:, :, HD:HD + 1], 1.0)

            # ---- attention ---------------------------------------------------
            if stage == "proj":
                n_pairs = 0
            elif stage == "attn1":
                n_pairs = 1
            else:
                n_pairs = 4
            for p in range(n_pairs):
                for qi in range(4 if stage != "attn1" else 1):
                    nk = 4 * (qi + 1)
                    qsl = slice(qi * 512, (qi + 1) * 512)
                    pso = [ps_o.tile([65, 512], F32, tag=f"o{hi}", name=f"pso{hi}") for hi in range(2)]
                    for jg in range(nk // 2):
                        js = (2 * jg, 2 * jg + 1)
                        ps = ps_s.tile([128, 2048], F32, name="ps_s")
                        for slot, j in enumerate(js):
                            for hi in range(2):
                                bidx = slot * 2 + hi
                                hh = slice(hi * 64, (hi + 1) * 64)
                                nc.tensor.matmul(
                                    ps[:, bidx * 512:(bidx + 1) * 512],
                                    lhsT=qkT[4 + p][hh, j * 128:(j + 1) * 128],
                                    rhs=qkT[p][hh, qsl],
                                    start=True, stop=True,
                                    tile_position=(64 * hi, 0),
                                )
                        expt = exps_pool.tile([128, 2048], BF16, tag="exps", name="expt")
                        nc.scalar.activation(
                            expt[:], ps[:],
                            mybir.ActivationFunctionType.Exp,
                            scale=float(1.0 / np.sqrt(HD)),
                        )
                        for slot, j in enumerate(js):
                            r = j - 4 * qi
                            if r >= 0:      # diagonal k-tile: apply causal mask
                                for hi in range(2):
                                    bidx = slot * 2 + hi
                                    bsl = slice(bidx * 512, (bidx + 1) * 512)
                                    nc.vector.tensor_mul(
                                        expt[:, bsl], expt[:, bsl],
                                        masks[:, r * 512:(r + 1) * 512],
                                    )
                        for slot, j in enumerate(js):
                            for hi in range(2):
                                bidx = slot * 2 + hi
                                hl = 2 * p + hi
                                nc.tensor.matmul(
                                    pso[hi][:],
                                    lhsT=Vt[j][:, hl * 65:hl * 65 + 65],
                                    rhs=expt[:, bidx * 512:(bidx + 1) * 512],
                                    start=(jg == 0 and slot == 0),
                                    stop=(jg == nk // 2 - 1 and slot == 1),
                                )
                    # normalize: O^T[hd, q] / rowsum (ones row of pso)
                    for hi in range(2):
                        rec = small.tile([1, 512], F32, tag="rec", name="rec")
                        nc.vector.reciprocal(rec[:], pso[hi][64:65, :])
                        bc = small.tile([64, 512], F32, tag="bc", name="bc")
                        nc.gpsimd.partition_broadcast(bc[:], rec[:], channels=64)
                        if hi == 0:
                            nc.vector.tensor_mul(
                                OTn[p][0:64, qsl], pso[hi][0:64, :], bc[:])
                        else:
                            tmp = small.tile([64, 512], BF16, tag="tmp", name="tmp")
                            nc.vector.tensor_mul(tmp[:], pso[hi][0:64, :], bc[:])
                            nc.sync.dma_start(out=OTn[p][64:128, qsl], in_=tmp[:])

            # ---- partial out-projection + pair ReduceScatter -----------------
            if stage in ("proj", "attn1", "attn"):
                # debug: dump a staged tile into y and stop here
                dbg = ystage.tile([128, D], F32, tag="rb", name="dbg")
                src = OTn[0][:, 0:D] if stage != "proj" else qkT[0][:, 0:D]
                nc.vector.tensor_copy(dbg[:], src)
                nc.sync.dma_start(out=y_d[0:128, :], in_=dbg[:])
                dbg2 = ystage.tile([128, D], F32, tag="rb2", name="dbg2")
                if stage == "proj":
                    nc.vector.tensor_copy(dbg2[:], qkT[4][:, 0:D])
                else:
                    nc.vector.tensor_copy(dbg2[:, 0:520], Vt[0][:, :])
                nc.sync.dma_start(out=y_d[128:256, :], in_=dbg2[:])
            else:
                y_part = dram.tile([L, D], F32)
                y_rs = dram.tile([LH, D], F32)
                for lb in range(16 if stage != "rsonly" else 0):
                    for nh in range(2):
                        ps = ps_op.tile([128, 512], F32, name="ps_proj")
                        for p in range(4):
                            nc.tensor.matmul(
                                ps[:],
                                lhsT=OTn[p][:, lb * 128:(lb + 1) * 128],
                                rhs=woT[p][:, nh * 512:(nh + 1) * 512],
                                start=(p == 0), stop=(p == 3),
                            )
                        yb = ystage.tile([128, 512], F32, tag="yb", name="yb")
                        nc.vector.tensor_copy(yb[:], ps[:])
                        nc.sync.dma_start(
                            out=y_part[lb * 128:(lb + 1) * 128,
                                       nh * 512:(nh + 1) * 512],
                            in_=yb[:],
                        )
                if dbg_d is not None:
                    for p in range(4):
                        for half in range(2):
                            dt_ = ystage.tile([128, L // 2], F32, tag="yb",
                                              name="dbgot")
                            nc.vector.tensor_copy(
                                dt_[:], OTn[p][:, half * (L // 2):(half + 1) * (L // 2)])
                            nc.sync.dma_start(
                                out=dbg_d[p * 128:(p + 1) * 128,
                                          half * (L // 2):(half + 1) * (L // 2)],
                                in_=dt_[:])
                if stage not in ("norsc", "norsc_dbg"):
                    nc.gpsimd.collective_compute(
                        "ReduceScatter",
                        mybir.AluOpType.add,
                        replica_groups=[[0, 1], [2, 3], [4, 5], [6, 7]],
                        ins=[y_part.opt()],
                        outs=[y_rs.opt()],
                    )
                rb_src = y_rs if stage not in ("norsc", "norsc_dbg") else y_part
                # ---- read back own half, add bo, write output ----------------
                for lb in range(8):
                    t = ystage.tile([128, D], F32, tag="rb", name="rb")
                    nc.sync.dma_start(out=t[:],
                                      in_=rb_src[lb * 128:(lb + 1) * 128, :])
                    t2 = ystage.tile([128, D], F32, tag="rb2", name="rb2")
                    nc.vector.tensor_add(t2[:], t[:], bob[:])
                    nc.sync.dma_start(out=y_d[lb * 128:(lb + 1) * 128, :],
                                      in_=t2[:])

    nc.compile()
    return nc


def _prep_core_inputs(c, x, Wqkv, bqkv, Wo, bo, masks_np):
    b, g = c // 2, c % 2
    qs = slice(g * GD, (g + 1) * GD)
    ks = slice(D + g * GD, D + (g + 1) * GD)
    vs = slice(2 * D + g * GD, 2 * D + (g + 1) * GD)
    Wc = np.concatenate([Wqkv[qs], Wqkv[ks], Wqkv[vs]], axis=0)
    return {
        "xT": np.ascontiguousarray(x[b].T).astype(bf16),
        "wT": np.ascontiguousarray(Wc.T).astype(bf16),
        "woT": np.ascontiguousarray(Wo[:, g * GD:(g + 1) * GD].T).astype(bf16),
        "bqk": np.concatenate([bqkv[qs], bqkv[ks]]).astype(np.float32).reshape(2 * GD, 1),
        "bv": np.tile(bqkv[vs].astype(np.float32), (128, 1)),
        "bob": np.tile(bo.astype(np.float32), (128, 1)),
        "masks": masks_np,
    }


def _masks_np():
    m = np.zeros((128, 4 * 512), dtype=bf16)
    kk = np.arange(128)[:, None]
    qq = np.arange(512)[None, :]
    for r in range(4):
        m[:, r * 512:(r + 1) * 512] = (qq >= kk + 128 * r).astype(bf16)
    return m


def _run(inputs, trace=False):
    if "nc" not in _CACHE:
        _CACHE["nc"] = _build_nc()
    nc = _CACHE["nc"]
    x = np.asarray(inputs["x"], dtype=np.float32)
    Wqkv = np.asarray(inputs["Wqkv"], dtype=np.float32)
    bqkv = np.asarray(inputs["bqkv"], dtype=np.float32)
    Wo = np.asarray(inputs["Wo"], dtype=np.float32)
    bo = np.asarray(inputs["bo"], dtype=np.float32)
    masks_np = _masks_np()
    in_maps = [_prep_core_inputs(c, x, Wqkv, bqkv, Wo, bo, masks_np)
               for c in range(8)]
    res = run_bass_kernel_spmd(nc, in_maps, core_ids=list(range(8)), trace=trace)
    out = np.empty((B, L, D), dtype=np.float32)
    for b in range(B):
        out[b, :LH] = res.results[2 * b]["y"]
        out[b, LH:] = res.results[2 * b + 1]["y"]
    return out, res


def kernel(x, mask, Wqkv, bqkv, Wo, bo):
    out, _ = _run({"x": x, "mask": mask, "Wqkv": Wqkv, "bqkv": bqkv,
                   "Wo": Wo, "bo": bo})
    return out


def kernel_traced(x, mask, Wqkv, bqkv, Wo, bo):
    return _run({"x": x, "mask": mask, "Wqkv": Wqkv, "bqkv": bqkv,
                 "Wo": Wo, "bo": bo}, trace=True)



# revision 7
# speedup vs baseline: 1.5523x; 1.5523x over previous
"""Causal multi-head attention block (QKV proj + SDPA + out proj) on 8 TRN2 cores.

Sharding: batch (4) x head-group (2 groups of 8 heads). Core c handles batch
c//2, heads [g*8, g*8+8) with g = c%2.

Pipelined structure (v2): q-block (qi) outer loop. Per qi, the 4 head-pairs run
attention back-to-back with a lag-4 S->EXP->PV software pipeline so the tensor
engine never drains; QKV projection chains for the NEXT q-block are emitted as
filler between pairs; the partial out-projection for q-block qi is emitted right
after its last pair and reduced across the core pair with a chunked fp32
ReduceScatter (2 sub-chunks of 256 q each) writing directly into the output
DRAM tensor (bias pre-folded as bo/2 into each partial).

All heavy matmuls are bf16 with fp32 PSUM accumulation. Hardcoded shapes per
the problem spec: x [4, 2048, 1024], 16 heads, head_dim 64.
"""
import sys

if '/opt/trn_rl_repo' not in sys.path:
    sys.path.insert(0, '/opt/trn_rl_repo')

import numpy as np
import ml_dtypes

import concourse.bass as bass
import concourse.mybir as mybir
from concourse import bacc
from concourse.bass_utils import run_bass_kernel_spmd
from concourse.tile import TileContext

bf16 = ml_dtypes.bfloat16
F32 = mybir.dt.float32
BF16 = mybir.dt.bfloat16

B, L, D, H, HD = 4, 2048, 1024, 16, 64
HPC = 8           # heads per core
GD = HPC * HD     # 512 dims per head-group
QB = 512          # query block
LAG = 4           # S->PV software pipeline depth (in k-tiles)

_CACHE = {}


def _build_nc():
    nc = bacc.Bacc("TRN2", target_bir_lowering=False, debug=False, num_devices=8)

    xT_d = nc.dram_tensor("xT", [D, L], BF16, kind="ExternalInput").ap()
    wT_d = nc.dram_tensor("wT", [D, 3 * GD], BF16, kind="ExternalInput").ap()
    woT_d = nc.dram_tensor("woT", [GD, D], BF16, kind="ExternalInput").ap()
    bqk_d = nc.dram_tensor("bqk", [2 * GD, 1], F32, kind="ExternalInput").ap()
    bv_d = nc.dram_tensor("bv", [128, GD], F32, kind="ExternalInput").ap()
    bob2_d = nc.dram_tensor("bob2", [128, D], F32, kind="ExternalInput").ap()
    masks_d = nc.dram_tensor("masks", [128, 4 * 512], BF16, kind="ExternalInput").ap()
    y_d = nc.dram_tensor("y", [1024, D], F32, kind="ExternalOutput").ap()

    with TileContext(nc) as tc:
        with (
            tc.tile_pool(name="persist", bufs=1) as persist,
            tc.tile_pool(name="exps", bufs=12) as exps_pool,
            tc.tile_pool(name="otn", bufs=2) as otn_pool,
            tc.tile_pool(name="small", bufs=3) as small,
            tc.tile_pool(name="ystage", bufs=4) as ystage,
            tc.tile_pool(name="ps_s", bufs=2, space="PSUM") as ps_s,
            tc.tile_pool(name="ps_o", bufs=1, space="PSUM") as ps_o,
            tc.tile_pool(name="ps_op", bufs=2, space="PSUM") as ps_op,
            tc.tile_pool(name="dram", bufs=1, space="DRAM") as dram,
        ):
            # ---- persistent SBUF tensors -------------------------------------
            xT = [persist.tile([128, L], BF16, tag=f"xT{c}", name=f"xT{c}") for c in range(8)]
            wT = [persist.tile([128, 3 * GD], BF16, tag=f"wT{c}", name=f"wT{c}") for c in range(8)]
            qkT = [persist.tile([128, L], BF16, tag=f"qkT{i}", name=f"qkT{i}") for i in range(8)]
            Vt = [persist.tile([128, HPC * (HD + 1)], BF16, tag=f"V{i}", name=f"V{i}")
                  for i in range(16)]
            woT = [persist.tile([128, D], BF16, tag=f"woT{p}", name=f"woT{p}") for p in range(4)]
            bqk = persist.tile([128, 8], F32, tag="bqk")
            bv = persist.tile([128, GD], F32, tag="bv")
            bob2 = persist.tile([128, D], F32, tag="bob2")
            masks = persist.tile([128, 4 * 512], BF16, tag="masks")

            ych = dram.tile([L, D], F32)
            yrs = dram.tile([1024, D], F32)

            # ---- input loads -------------------------------------------------
            nc.sync.dma_start(out=masks, in_=masks_d[:, :])
            for dt in range(8):
                nc.sync.dma_start(out=bqk[:, dt:dt + 1],
                                  in_=bqk_d[dt * 128:(dt + 1) * 128, :])
            nc.sync.dma_start(out=bv, in_=bv_d[:, :])
            for c in range(8):
                nc.sync.dma_start(out=wT[c], in_=wT_d[c * 128:(c + 1) * 128, :])
                nc.sync.dma_start(out=xT[c], in_=xT_d[c * 128:(c + 1) * 128, :])
            for p in range(4):
                nc.sync.dma_start(out=woT[p], in_=woT_d[p * 128:(p + 1) * 128, :])
            nc.sync.dma_start(out=bob2, in_=bob2_d[:, :])

            # ---- projection chain emitters -----------------------------------
            def qk_chain(dt, lsb):
                ps = ps_op.tile([128, 512], F32, name="ps_proj", tag="mm")
                for c in range(8):
                    nc.tensor.matmul(
                        ps[:],
                        lhsT=wT[c][:, dt * 128:(dt + 1) * 128],
                        rhs=xT[c][:, lsb * 512:(lsb + 1) * 512],
                        start=(c == 0), stop=(c == 7),
                    )
                nc.scalar.activation(
                    qkT[dt][:, lsb * 512:(lsb + 1) * 512], ps[:],
                    mybir.ActivationFunctionType.Identity,
                    bias=bqk[:, dt:dt + 1],
                )

            def v_chain(lb):
                ps = ps_op.tile([128, 512], F32, name="ps_proj", tag="mm")
                for c in range(8):
                    nc.tensor.matmul(
                        ps[:],
                        lhsT=xT[c][:, lb * 128:(lb + 1) * 128],
                        rhs=wT[c][:, 1024:1536],
                        start=(c == 0), stop=(c == 7),
                    )
                v_grp = Vt[lb][:].rearrange("p (h c) -> p h c", c=HD + 1)
                nc.vector.tensor_add(
                    v_grp[:, :, 0:HD],
                    ps[:].rearrange("p (h c) -> p h c", c=HD),
                    bv[:].rearrange("p (h c) -> p h c", c=HD),
                )
                nc.vector.memset(v_grp[:, :, HD:HD + 1], 1.0)

            def proj_block_chains(lsb):
                # 12 chains for this L-column block, ordered so qk lands first
                chains = []
                for dt in range(4):
                    chains.append(lambda dt=dt: qk_chain(dt, lsb))
                    chains.append(lambda dt=dt: qk_chain(4 + dt, lsb))
                for i in range(4):
                    chains.append(lambda i=i: v_chain(4 * lsb + i))
                return chains

            # ---- attention pair emitter (lag-LAG pipeline) -------------------
            cur_otn = {}

            def emit_S(p, qi, j):
                qsl = slice(qi * QB, (qi + 1) * QB)
                ps = ps_s.tile([128, 1024], F32, name="ps_sc", tag="s")
                for hi in range(2):
                    hh = slice(hi * 64, (hi + 1) * 64)
                    nc.tensor.matmul(
                        ps[:, hi * 512:(hi + 1) * 512],
                        lhsT=qkT[4 + p][hh, j * 128:(j + 1) * 128],
                        rhs=qkT[p][hh, qsl],
                        start=True, stop=True,
                        tile_position=(64 * hi, 0),
                    )
                expt = exps_pool.tile([128, 1024], BF16, tag="e", name="expt")
                nc.scalar.activation(
                    expt[:], ps[:],
                    mybir.ActivationFunctionType.Exp,
                    scale=float(1.0 / np.sqrt(HD)),
                )
                r = j - 4 * qi
                if r >= 0:      # diagonal k-tile: apply causal mask
                    for hi in range(2):
                        bsl = slice(hi * 512, (hi + 1) * 512)
                        nc.vector.tensor_mul(
                            expt[:, bsl], expt[:, bsl],
                            masks[:, r * 512:(r + 1) * 512],
                        )
                return expt

            def attn_pair(p, qi):
                js = list(range(4 * (qi + 1)))
                pso = [ps_o.tile([65, 512], F32, tag=f"o{hi}", name=f"pso{hi}")
                       for hi in range(2)]
                expts = {}

                def emit_PV(j):
                    for hi in range(2):
                        hl = 2 * p + hi
                        nc.tensor.matmul(
                            pso[hi][:],
                            lhsT=Vt[j][:, hl * 65:hl * 65 + 65],
                            rhs=expts[j][:, hi * 512:(hi + 1) * 512],
                            start=(j == js[0]), stop=(j == js[-1]),
                        )

                for idx, j in enumerate(js):
                    expts[j] = emit_S(p, qi, j)
                    if idx >= LAG:
                        emit_PV(js[idx - LAG])
                for j in js[-LAG:] if len(js) >= LAG else js:
                    emit_PV(j)

                # normalize: O^T[hd, q] / rowsum (ones row of pso)
                qsl = slice(qi * QB, (qi + 1) * QB)
                otn_t = otn_pool.tile([128, 512], BF16, tag=f"otn{p}",
                                      name=f"otn{p}")
                cur_otn[p] = otn_t
                for hi in range(2):
                    rec = small.tile([1, 512], F32, tag="rec", name="rec")
                    nc.vector.reciprocal(rec[:], pso[hi][64:65, :])
                    bc = small.tile([64, 512], F32, tag="bc", name="bc")
                    nc.gpsimd.partition_broadcast(bc[:], rec[:], channels=64)
                    if hi == 0:
                        nc.vector.tensor_mul(
                            otn_t[0:64, :], pso[hi][0:64, :], bc[:])
                    else:
                        tmp = small.tile([64, 512], BF16, tag="tmp", name="tmp")
                        nc.vector.tensor_mul(tmp[:], pso[hi][0:64, :], bc[:])
                        nc.sync.dma_start(out=otn_t[64:128, :], in_=tmp[:])

            # ---- partial out-projection + chunked pair ReduceScatter ---------
            def outproj_rs(qi):
                for sc in range(2):
                    for lb2 in range(2):
                        off = (sc * 2 + lb2) * 128
                        for nh in range(2):
                            ps = ps_op.tile([128, 512], F32, name="ps_proj",
                                            tag="mm")
                            for p in range(4):
                                nc.tensor.matmul(
                                    ps[:],
                                    lhsT=cur_otn[p][:, off:off + 128],
                                    rhs=woT[p][:, nh * 512:(nh + 1) * 512],
                                    start=(p == 0), stop=(p == 3),
                                )
                            yb = ystage.tile([128, 512], F32, tag="yb",
                                             name="yb")
                            nc.vector.tensor_add(
                                yb[:], ps[:], bob2[:, nh * 512:(nh + 1) * 512])
                            nc.sync.dma_start(
                                out=ych[qi * 512 + off:qi * 512 + off + 128,
                                        nh * 512:(nh + 1) * 512],
                                in_=yb[:],
                            )
                    rows = slice(qi * 512 + sc * 256, qi * 512 + sc * 256 + 256)
                    orows = slice(qi * 256 + sc * 128, qi * 256 + sc * 128 + 128)
                    nc.gpsimd.collective_compute(
                        "ReduceScatter",
                        mybir.AluOpType.add,
                        replica_groups=[[0, 1], [2, 3], [4, 5], [6, 7]],
                        ins=[ych[rows, :].opt()],
                        outs=[yrs[orows, :].opt()],
                    )
                    nc.sync.dma_start(out=y_d[orows, :], in_=yrs[orows, :])

            # ---- main emission schedule --------------------------------------
            for ch in proj_block_chains(0):
                ch()
            fillers = {qi: proj_block_chains(qi + 1) for qi in range(3)}
            for qi in range(4):
                for p in range(4):
                    attn_pair(p, qi)
                    if qi < 3:
                        for ch in fillers[qi][p * 3:(p + 1) * 3]:
                            ch()
                outproj_rs(qi)

    nc.compile()
    return nc


def _prep_core_inputs(c, x, Wqkv, bqkv, Wo, bo, masks_np):
    b, g = c // 2, c % 2
    qs = slice(g * GD, (g + 1) * GD)
    ks = slice(D + g * GD, D + (g + 1) * GD)
    vs = slice(2 * D + g * GD, 2 * D + (g + 1) * GD)
    Wc = np.concatenate([Wqkv[qs], Wqkv[ks], Wqkv[vs]], axis=0)
    return {
        "xT": np.ascontiguousarray(x[b].T).astype(bf16),
        "wT": np.ascontiguousarray(Wc.T).astype(bf16),
        "woT": np.ascontiguousarray(Wo[:, g * GD:(g + 1) * GD].T).astype(bf16),
        "bqk": np.concatenate([bqkv[qs], bqkv[ks]]).astype(np.float32).reshape(2 * GD, 1),
        "bv": np.tile(bqkv[vs].astype(np.float32), (128, 1)),
        "bob2": np.tile(0.5 * bo.astype(np.float32), (128, 1)),
        "masks": masks_np,
    }


def _masks_np():
    m = np.zeros((128, 4 * 512), dtype=bf16)
    kk = np.arange(128)[:, None]
    qq = np.arange(512)[None, :]
    for r in range(4):
        m[:, r * 512:(r + 1) * 512] = (qq >= kk + 128 * r).astype(bf16)
    return m


def _run(inputs, trace=False):
    if "nc" not in _CACHE:
        _CACHE["nc"] = _build_nc()
    nc = _CACHE["nc"]
    x = np.asarray(inputs["x"], dtype=np.float32)
    Wqkv = np.asarray(inputs["Wqkv"], dtype=np.float32)
    bqkv = np.asarray(inputs["bqkv"], dtype=np.float32)
    Wo = np.asarray(inputs["Wo"], dtype=np.float32)
    bo = np.asarray(inputs["bo"], dtype=np.float32)
    masks_np = _masks_np()
    in_maps = [_prep_core_inputs(c, x, Wqkv, bqkv, Wo, bo, masks_np)
               for c in range(8)]
    res = run_bass_kernel_spmd(nc, in_maps, core_ids=list(range(8)), trace=trace)
    out = np.empty((B, L, D), dtype=np.float32)
    for b in range(B):
        for g in range(2):
            yc = res.results[2 * b + g]["y"]
            for qi in range(4):
                for sc in range(2):
                    dst = qi * 512 + sc * 256 + g * 128
                    src = qi * 256 + sc * 128
                    out[b, dst:dst + 128] = yc[src:src + 128]
    return out, res


def kernel(x, mask, Wqkv, bqkv, Wo, bo):
    out, _ = _run({"x": x, "mask": mask, "Wqkv": Wqkv, "bqkv": bqkv,
                   "Wo": Wo, "bo": bo})
    return out


def kernel_traced(x, mask, Wqkv, bqkv, Wo, bo):
    return _run({"x": x, "mask": mask, "Wqkv": Wqkv, "bqkv": bqkv,
                 "Wo": Wo, "bo": bo}, trace=True)
